# revision 4
# baseline (speedup 1.0000x reference)
"""Trainium2 Bass kernel for nn_CriterionLP (LP contrastive criterion loss).

Reference computation (B=2048 anchors, M=16384 supports, C=256, K=128 label
groups of G=128 supports each):
    sim   = (feats @ Fs.T) / TEMP                  [B, M]
    E     = exp(sim) grouped into K blocks of G    [B, K, G]
    pos   = exp(min sim over own-label block)      (one block per row)
    neg   = sum over other blocks of exp(max sim over block)
    loss  = mean_b( -log(pos/(pos+neg+eps) + eps) )

Sharding: support axis across 8 cores (16 groups / 2048 support rows per
core).  Each core loads the full feats (bf16, 1MB) plus its Fs shard (1MB),
computes per-block stats for its groups, and the per-row pos/neg partial
sums are combined with an on-device AllReduce; every core then computes the
identical final scalar loss.

Key tricks:
  - exp is monotonic: block min/max are taken on raw matmul scores; exp only
    runs on the [128, 256] block-stat arrays.
  - per-core row rotation puts each core's own-label (positive) rows into
    b-tiles 0..1, so the min-reduction only runs on 2 of 16 b-tiles with a
    core-uniform program; partial sums are un-rotated with a dynamic-slice
    DMA (offset = f(partition_id)) before the AllReduce.
  - PSUM -> SBUF bf16 copies on the scalar engine; block max via pairwise
    tensor_tensor max tree on the vector engine (bf16 2x mode; plain
    tensor_reduce only runs at 1x).
"""

import numpy as np
import ml_dtypes

import concourse.bass as bass
import concourse.bacc as bacc
import concourse.tile as tile
import concourse.mybir as mybir
from concourse.bass_utils import run_bass_kernel_spmd

F32 = mybir.dt.float32
BF16 = mybir.dt.bfloat16
AX = mybir.AxisListType
ALU = mybir.AluOpType
ACTF = mybir.ActivationFunctionType

TEMP = 0.05
EPS = 1e-6
B, C = 2048, 256
NCORES = 8
KTOT, G = 128, 128          # label groups, supports per group
MLOC = 2048                 # support rows per core
KLOC = KTOT // NCORES       # groups per core (16)
NBT = B // 128              # b tiles of 128 rows (16)
NMT = MLOC // 512           # m tiles of 512 cols (4)

_PROG_CACHE = {}
LAST_RESULT = None          # BassKernelResults of the most recent run


def _tree(nc, pool, cpv, out16, op):
    """Block reduce [128, 16, 128] -> [128, 16] via pairwise TT ops (bf16 2x)."""
    t1 = pool.tile([128, KLOC, 64], BF16, name="t1", tag="t1", bufs=3)
    nc.vector.tensor_tensor(t1[:], cpv[:, :, 0:64], cpv[:, :, 64:128], op)
    t2 = pool.tile([128, KLOC, 32], BF16, name="t2", tag="t2", bufs=3)
    nc.vector.tensor_tensor(t2[:], t1[:, :, 0:32], t1[:, :, 32:64], op)
    t3 = pool.tile([128, KLOC, 16], BF16, name="t3", tag="t3", bufs=3)
    nc.vector.tensor_tensor(t3[:], t2[:, :, 0:16], t2[:, :, 16:32], op)
    t4 = pool.tile([128, KLOC, 8], BF16, name="t4", tag="t4", bufs=3)
    nc.vector.tensor_tensor(t4[:], t3[:, :, 0:8], t3[:, :, 8:16], op)
    nc.vector.tensor_reduce(out16, t4[:], axis=AX.X, op=op)


def _build(min_bts, rotate):
    key = (tuple(sorted(min_bts)), rotate)
    if key in _PROG_CACHE:
        return _PROG_CACHE[key]

    nc = bacc.Bacc("TRN2", target_bir_lowering=False, debug=False,
                   num_devices=NCORES)
    ftd = nc.dram_tensor("featsT", [2, 128, B], BF16, kind="ExternalInput")
    fsd = nc.dram_tensor("fsT", [2, 128, MLOC], BF16, kind="ExternalInput")
    mpd = nc.dram_tensor("mpos", [128, NBT * KLOC], F32, kind="ExternalInput")
    mnd = nc.dram_tensor("mneg", [128, NBT * KLOC], F32, kind="ExternalInput")
    lossd = nc.dram_tensor("loss", [1, 1], F32, kind="ExternalOutput")

    with tile.TileContext(nc) as tc:
        with (
            tc.tile_pool(name="wpool", bufs=1) as wp,
            tc.tile_pool(name="cpool", bufs=3) as cpp,
            tc.tile_pool(name="tpool", bufs=3) as trp,
            tc.tile_pool(name="spool", bufs=1) as stp,
            tc.tile_pool(name="pspool", bufs=2, space="PSUM") as psp,
            tc.tile_pool(name="drpool", bufs=1, space="DRAM") as drp,
        ):
            # --- input loads: fs via HWDGE(SP), ft via SWDGE(gpsimd) so the
            # two streams drain in parallel and compute can start early ---
            fs_sb = []
            for mt in range(NMT):
                t = wp.tile([128, 2, 512], BF16, name=f"fs{mt}", tag=f"fs{mt}")
                for ch in range(2):
                    nc.sync.dma_start(t[:, ch, :], fsd[ch, :, mt * 512:(mt + 1) * 512])
                fs_sb.append(t)
            ft_sb = []
            for bt in range(NBT):
                t = wp.tile([128, 2, 128], BF16, name=f"ft{bt}", tag=f"ft{bt}")
                for ch in range(2):
                    nc.gpsimd.dma_start(t[:, ch, :], ftd[ch, :, bt * 128:(bt + 1) * 128])
                ft_sb.append(t)
            mpos = stp.tile([128, NBT * KLOC], F32, name="mpos_sb")
            nc.gpsimd.dma_start(mpos[:], mpd[:, :])
            mneg = stp.tile([128, NBT * KLOC], F32, name="mneg_sb")
            nc.gpsimd.dma_start(mneg[:], mnd[:, :])

            # warm-up collective: absorbs the collectives entry barrier,
            # cross-core launch skew and the first ncfw trigger latency
            # while the matmul pipeline runs
            dmy = stp.tile([128, 1], F32, name="dmy")
            nc.vector.memset(dmy[:], 0.0)
            dmy_in = drp.tile([128, 1], F32, name="dmy_in")
            dmy_out = drp.tile([128, 1], F32, name="dmy_out", addr_space="Shared")
            nc.sync.dma_start(dmy_in[:, :], dmy[:])
            nc.gpsimd.collective_compute(
                "AllReduce", ALU.add,
                replica_groups=[list(range(NCORES))],
                ins=[dmy_in[:, :].opt()],
                outs=[dmy_out[:, :].opt()],
            )

            blkmax = stp.tile([128, NBT * KLOC], BF16, name="blkmax")
            blkmin = stp.tile([128, NBT * KLOC], BF16, name="blkmin")
            # untouched min slots stay 0 -> exp(0)=1, masked to 0 by mpos
            nc.vector.memset(blkmin[:], 0.0)

            # --- main loop: matmul -> bf16 copy -> block max (+min) ---
            for bt in range(NBT):
                ps = psp.tile([128, MLOC], F32, name="ps", tag="ps")
                for ch in range(2):
                    for mt in range(NMT):
                        nc.tensor.matmul(
                            ps[:, mt * 512:(mt + 1) * 512],
                            ft_sb[bt][:, ch, :],
                            fs_sb[mt][:, ch, :],
                            start=(ch == 0),
                            stop=(ch == 1),
                        )
                cp = cpp.tile([128, MLOC], BF16, name="cp", tag="cp")
                nc.scalar.copy(cp[:], ps[:])
                cpv = cp.rearrange("p (k g) -> p k g", g=G)
                _tree(nc, trp, cpv, blkmax[:, bt * KLOC:(bt + 1) * KLOC], ALU.max)
                if bt in min_bts:
                    _tree(nc, trp, cpv, blkmin[:, bt * KLOC:(bt + 1) * KLOC], ALU.min)

            # --- epilogue: exp, masked sums ---
            emin = stp.tile([128, NBT * KLOC], F32, name="emin")
            nc.scalar.activation(emin[:], blkmin[:], ACTF.Exp, scale=1.0 / TEMP)
            emax = stp.tile([128, NBT * KLOC], F32, name="emax")
            nc.scalar.activation(emax[:], blkmax[:], ACTF.Exp, scale=1.0 / TEMP)
            prodp = stp.tile([128, NBT * KLOC], F32, name="prodp")
            nc.vector.tensor_mul(prodp[:], emin[:], mpos[:])
            prodn = stp.tile([128, NBT * KLOC], F32, name="prodn")
            nc.vector.tensor_mul(prodn[:], emax[:], mneg[:])

            # pack2: [p, t(=bt slot, doubled), side]; write once, duplicate.
            # bf16 payload halves the AllReduce transfer time.
            pack2 = stp.tile([128, 2 * NBT, 2], BF16, name="pack2")
            with nc.allow_low_precision("bf16 collective payload; 0.4% rel on sums is fine"):
                nc.vector.tensor_reduce(
                    pack2[:, 0:NBT, 0],
                    prodp.rearrange("p (t k) -> p t k", k=KLOC),
                    axis=AX.X, op=ALU.add)
                nc.vector.tensor_reduce(
                    pack2[:, 0:NBT, 1],
                    prodn.rearrange("p (t k) -> p t k", k=KLOC),
                    axis=AX.X, op=ALU.add)

            cc_in = drp.tile([128, NBT, 2], BF16, name="cc_in")
            cc_out = drp.tile([128, NBT, 2], BF16, name="cc_out", addr_space="Shared")
            if rotate:
                nc.vector.tensor_copy(pack2[:, NBT:2 * NBT, :], pack2[:, 0:NBT, :])
                # local bt -> global bt is a rotation by 2*pid; un-rotate by
                # reading a dynamic window of the doubled buffer
                pid = nc.partition_id(engines=[mybir.EngineType.SP])
                w = NBT - 2 * pid
                nc.sync.dma_start(cc_in[:, :, :], pack2[:, bass.ds(w, NBT), :])
            else:
                nc.sync.dma_start(cc_in[:, :, :], pack2[:, 0:NBT, :])

            nc.gpsimd.collective_compute(
                "AllReduce", ALU.add,
                replica_groups=[list(range(NCORES))],
                ins=[cc_in[:, :, :].opt()],
                outs=[cc_out[:, :, :].opt()],
            )

            red = stp.tile([128, NBT, 2], BF16, name="red")
            nc.sync.dma_start(red[:, :, :], cc_out[:, :, :])

            # --- final loss: -mean(log(pos/(pos+neg+eps)+eps)) ---
            pos = red[:, :, 0]
            neg = red[:, :, 1]
            den2 = stp.tile([128, NBT], F32, name="den2")
            nc.vector.scalar_tensor_tensor(
                den2[:], pos, float(EPS), neg, ALU.add, ALU.add)
            rec = stp.tile([128, NBT], F32, name="rec")
            nc.vector.reciprocal(rec[:], den2[:])
            ratio = stp.tile([128, NBT], F32, name="ratio")
            nc.vector.tensor_mul(ratio[:], pos, rec[:])
            lg = stp.tile([128, NBT], F32, name="lg")
            nc.scalar.activation(lg[:], ratio[:], ACTF.Ln, bias=float(EPS))
            # partition sum via matmul; fold -1/B into the ones vector
            ones = stp.tile([128, 1], F32, name="ones")
            nc.vector.memset(ones[:], -1.0 / B)
            pl = psp.tile([128, MLOC], F32, name="pl", tag="ps")
            nc.tensor.matmul(pl[:1, 0:NBT], ones[:], lg[:])
            lout = stp.tile([1, 1], F32, name="lout")
            nc.vector.tensor_reduce(lout[:], pl[:1, 0:NBT], axis=AX.X, op=ALU.add)
            nc.sync.dma_start(lossd[:, :], lout[:])

    nc.compile()
    _PROG_CACHE[key] = nc
    return nc


def kernel(feats, feats_s, labels, labels_s, topk, num_instances):
    global LAST_RESULT
    feats = np.asarray(feats, dtype=np.float32)
    feats_s = np.asarray(feats_s, dtype=np.float32)
    labels = np.asarray(labels).astype(np.int64).ravel()
    labels_s = np.asarray(labels_s).astype(np.int64).ravel()
    tk, ni = int(topk), int(num_instances)
    assert feats.shape == (B, C), feats.shape
    assert tk * ni == G and feats_s.shape == (B, tk, C)

    Fs = feats_s.reshape(-1, C)                       # [16384, 256]
    glab = labels_s.reshape(KTOT, G)[:, 0]            # label of each block

    # rotation is valid if each core's own-label rows are exactly the
    # contiguous global rows [256j, 256j+256)
    rotate = True
    for j in range(NCORES):
        own = np.isin(labels, glab[j * KLOC:(j + 1) * KLOC])
        want = np.zeros(B, dtype=bool)
        want[j * (B // NCORES):(j + 1) * (B // NCORES)] = True
        if not np.array_equal(own, want):
            rotate = False
            break
    min_bts = (0, 1) if rotate else tuple(range(NBT))

    nc = _build(min_bts, rotate)

    in_maps = []
    for j in range(NCORES):
        shift = (B // NCORES) * j
        f_loc = np.roll(feats, -shift, axis=0) if rotate else feats
        lab_loc = np.roll(labels, -shift) if rotate else labels
        ftT = np.ascontiguousarray(f_loc.T).reshape(2, 128, B)
        fsT = np.ascontiguousarray(Fs[j * MLOC:(j + 1) * MLOC].T).reshape(2, 128, MLOC)
        # masks in local (rotated) coords: [p, bt*KLOC + k]
        lab2 = lab_loc.reshape(NBT, 128)                        # [bt, p]
        gl_j = glab[j * KLOC:(j + 1) * KLOC]                    # [KLOC]
        mp = (lab2[:, :, None] == gl_j[None, None, :])          # [bt, p, k]
        mp = mp.transpose(1, 0, 2).reshape(128, NBT * KLOC)
        in_maps.append({
            "featsT": ftT.astype(ml_dtypes.bfloat16),
            "fsT": fsT.astype(ml_dtypes.bfloat16),
            "mpos": mp.astype(np.float32),
            "mneg": (~mp).astype(np.float32),
        })

    LAST_RESULT = run_bass_kernel_spmd(nc, in_maps, core_ids=list(range(NCORES)))
    loss = LAST_RESULT.results[0]["loss"][0, 0]
    return np.asarray(loss, dtype=np.float32).reshape(())


# revision 5
# speedup vs baseline: 1.2645x; 1.2645x over previous
"""Trainium2 Bass kernel for nn_CriterionLP (LP contrastive criterion loss).

Reference computation (B=2048 anchors, M=16384 supports, C=256, K=128 label
groups of G=128 supports each):
    sim   = (feats @ Fs.T) / TEMP                  [B, M]
    E     = exp(sim) grouped into K blocks of G    [B, K, G]
    pos   = exp(min sim over own-label block)      (one block per row)
    neg   = sum over other blocks of exp(max sim over block)
    loss  = mean_b( -log(pos/(pos+neg+eps) + eps) )

Sharding: support axis across 8 cores (16 groups / 2048 support rows per
core).  Each core loads the full feats (bf16, 1MB) plus its Fs shard (1MB),
computes per-block stats for its groups, and the per-row pos/neg partial
sums are combined with an on-device AllReduce; every core then computes the
identical final scalar loss.

Key tricks:
  - exp is monotonic: block min/max are taken on raw matmul scores; exp only
    runs on the [128, 256] block-stat arrays.
  - per-core row rotation puts each core's own-label (positive) rows into
    b-tiles 0..1, so the min-reduction only runs on 2 of 16 b-tiles with a
    core-uniform program; partial sums are un-rotated with a dynamic-slice
    DMA (offset = f(partition_id)) before the AllReduce.
  - PSUM -> SBUF bf16 copies on the scalar engine; block max via pairwise
    tensor_tensor max tree on the vector engine (bf16 2x mode; plain
    tensor_reduce only runs at 1x).
"""

import numpy as np
import ml_dtypes

import concourse.bass as bass
import concourse.bacc as bacc
import concourse.tile as tile
import concourse.mybir as mybir
from concourse.bass_utils import run_bass_kernel_spmd

F32 = mybir.dt.float32
BF16 = mybir.dt.bfloat16
AX = mybir.AxisListType
ALU = mybir.AluOpType
ACTF = mybir.ActivationFunctionType

TEMP = 0.05
EPS = 1e-6
B, C = 2048, 256
NCORES = 8
KTOT, G = 128, 128          # label groups, supports per group
MLOC = 2048                 # support rows per core
KLOC = KTOT // NCORES       # groups per core (16)
NBT = B // 128              # b tiles of 128 rows (16)
NMT = MLOC // 512           # m tiles of 512 cols (4)

_PROG_CACHE = {}
LAST_RESULT = None          # BassKernelResults of the most recent run


def _tree(nc, pool, cpv, out16, op):
    """Block reduce [128, 16, 128] -> [128, 16] via pairwise TT ops (bf16 2x)."""
    t1 = pool.tile([128, KLOC, 64], BF16, name="t1", tag="t1", bufs=3)
    nc.vector.tensor_tensor(t1[:], cpv[:, :, 0:64], cpv[:, :, 64:128], op)
    t2 = pool.tile([128, KLOC, 32], BF16, name="t2", tag="t2", bufs=3)
    nc.vector.tensor_tensor(t2[:], t1[:, :, 0:32], t1[:, :, 32:64], op)
    t3 = pool.tile([128, KLOC, 16], BF16, name="t3", tag="t3", bufs=3)
    nc.vector.tensor_tensor(t3[:], t2[:, :, 0:16], t2[:, :, 16:32], op)
    t4 = pool.tile([128, KLOC, 8], BF16, name="t4", tag="t4", bufs=3)
    nc.vector.tensor_tensor(t4[:], t3[:, :, 0:8], t3[:, :, 8:16], op)
    nc.vector.tensor_reduce(out16, t4[:], axis=AX.X, op=op)


def _build(min_bts, rotate):
    key = (tuple(sorted(min_bts)), rotate)
    if key in _PROG_CACHE:
        return _PROG_CACHE[key]

    nc = bacc.Bacc("TRN2", target_bir_lowering=False, debug=False,
                   num_devices=NCORES)
    ftd = nc.dram_tensor("featsT", [2, 128, B], BF16, kind="ExternalInput")
    fsd = nc.dram_tensor("fsT", [2, 128, MLOC], BF16, kind="ExternalInput")
    mpd = nc.dram_tensor("mpos", [128, NBT * KLOC], F32, kind="ExternalInput")
    mnd = nc.dram_tensor("mneg", [128, NBT * KLOC], F32, kind="ExternalInput")
    lossd = nc.dram_tensor("loss", [1, 1], F32, kind="ExternalOutput")

    with tile.TileContext(nc) as tc:
        with (
            tc.tile_pool(name="wpool", bufs=1) as wp,
            tc.tile_pool(name="cpool", bufs=3) as cpp,
            tc.tile_pool(name="tpool", bufs=3) as trp,
            tc.tile_pool(name="spool", bufs=1) as stp,
            tc.tile_pool(name="pspool", bufs=2, space="PSUM") as psp,
            tc.tile_pool(name="drpool", bufs=1, space="DRAM") as drp,
        ):
            # --- input loads: fs via HWDGE(SP), ft via SWDGE(gpsimd) so the
            # two streams drain in parallel and compute can start early ---
            fs_sb = []
            for mt in range(NMT):
                t = wp.tile([128, 2, 512], BF16, name=f"fs{mt}", tag=f"fs{mt}")
                for ch in range(2):
                    nc.sync.dma_start(t[:, ch, :], fsd[ch, :, mt * 512:(mt + 1) * 512])
                fs_sb.append(t)
            ft_sb = []
            for bt in range(NBT):
                t = wp.tile([128, 2, 128], BF16, name=f"ft{bt}", tag=f"ft{bt}")
                for ch in range(2):
                    nc.gpsimd.dma_start(t[:, ch, :], ftd[ch, :, bt * 128:(bt + 1) * 128])
                ft_sb.append(t)
            mpos = stp.tile([128, NBT * KLOC], F32, name="mpos_sb")
            nc.gpsimd.dma_start(mpos[:], mpd[:, :])
            mneg = stp.tile([128, NBT * KLOC], F32, name="mneg_sb")
            nc.gpsimd.dma_start(mneg[:], mnd[:, :])

            # warm-up collective: absorbs the collectives entry barrier,
            # cross-core launch skew and the first ncfw trigger latency
            # while the matmul pipeline runs
            dmy = stp.tile([128, 1], F32, name="dmy")
            nc.vector.memset(dmy[:], 0.0)
            dmy_in = drp.tile([128, 1], F32, name="dmy_in")
            dmy_out = drp.tile([128, 1], F32, name="dmy_out", addr_space="Shared")
            nc.sync.dma_start(dmy_in[:, :], dmy[:])
            nc.gpsimd.collective_compute(
                "AllReduce", ALU.add,
                replica_groups=[list(range(NCORES))],
                ins=[dmy_in[:, :].opt()],
                outs=[dmy_out[:, :].opt()],
            )

            blkmax = stp.tile([128, NBT * KLOC], BF16, name="blkmax")
            blkmin = stp.tile([128, NBT * KLOC], BF16, name="blkmin")
            # untouched min slots stay 0 -> exp(0)=1, masked to 0 by mpos
            nc.vector.memset(blkmin[:], 0.0)

            # --- main loop: matmul -> bf16 copy -> block max (+min) ---
            for bt in range(NBT):
                ps = psp.tile([128, MLOC], F32, name="ps", tag="ps")
                for ch in range(2):
                    for mt in range(NMT):
                        nc.tensor.matmul(
                            ps[:, mt * 512:(mt + 1) * 512],
                            ft_sb[bt][:, ch, :],
                            fs_sb[mt][:, ch, :],
                            start=(ch == 0),
                            stop=(ch == 1),
                        )
                cp = cpp.tile([128, MLOC], BF16, name="cp", tag="cp")
                nc.scalar.copy(cp[:], ps[:])
                cpv = cp.rearrange("p (k g) -> p k g", g=G)
                _tree(nc, trp, cpv, blkmax[:, bt * KLOC:(bt + 1) * KLOC], ALU.max)
                if bt in min_bts:
                    _tree(nc, trp, cpv, blkmin[:, bt * KLOC:(bt + 1) * KLOC], ALU.min)

            # --- epilogue: exp, masked sums ---
            emin = stp.tile([128, NBT * KLOC], F32, name="emin")
            nc.scalar.activation(emin[:], blkmin[:], ACTF.Exp, scale=1.0 / TEMP)
            emax = stp.tile([128, NBT * KLOC], F32, name="emax")
            nc.scalar.activation(emax[:], blkmax[:], ACTF.Exp, scale=1.0 / TEMP)
            prodp = stp.tile([128, NBT * KLOC], F32, name="prodp")
            nc.vector.tensor_mul(prodp[:], emin[:], mpos[:])
            prodn = stp.tile([128, NBT * KLOC], F32, name="prodn")
            nc.vector.tensor_mul(prodn[:], emax[:], mneg[:])

            # pack2: [p, t(=bt slot, doubled), side]; write once, duplicate.
            # bf16 payload halves the AllReduce transfer time.
            pack2 = stp.tile([128, 2 * NBT, 2], BF16, name="pack2")
            with nc.allow_low_precision("bf16 collective payload; 0.4% rel on sums is fine"):
                nc.vector.tensor_reduce(
                    pack2[:, 0:NBT, 0],
                    prodp.rearrange("p (t k) -> p t k", k=KLOC),
                    axis=AX.X, op=ALU.add)
                nc.vector.tensor_reduce(
                    pack2[:, 0:NBT, 1],
                    prodn.rearrange("p (t k) -> p t k", k=KLOC),
                    axis=AX.X, op=ALU.add)

            cc_in = drp.tile([128, NBT, 2], BF16, name="cc_in")
            cc_out = drp.tile([128, NBT, 2], BF16, name="cc_out", addr_space="Shared")
            if rotate:
                nc.vector.tensor_copy(pack2[:, NBT:2 * NBT, :], pack2[:, 0:NBT, :])
                # local bt -> global bt is a rotation by 2*pid; un-rotate by
                # reading a dynamic window of the doubled buffer
                pid = nc.partition_id(engines=[mybir.EngineType.SP])
                w = NBT - 2 * pid
                nc.sync.dma_start(cc_in[:, :, :], pack2[:, bass.ds(w, NBT), :])
            else:
                nc.sync.dma_start(cc_in[:, :, :], pack2[:, 0:NBT, :])

            nc.gpsimd.collective_compute(
                "AllReduce", ALU.add,
                replica_groups=[list(range(NCORES))],
                ins=[cc_in[:, :, :].opt()],
                outs=[cc_out[:, :, :].opt()],
            )

            red = stp.tile([128, NBT, 2], BF16, name="red")
            nc.sync.dma_start(red[:, :, :], cc_out[:, :, :])

            # --- final loss: -mean(log(pos/(pos+neg+eps)+eps)) ---
            pos = red[:, :, 0]
            neg = red[:, :, 1]
            den2 = stp.tile([128, NBT], F32, name="den2")
            nc.vector.scalar_tensor_tensor(
                den2[:], pos, float(EPS), neg, ALU.add, ALU.add)
            rec = stp.tile([128, NBT], F32, name="rec")
            nc.vector.reciprocal(rec[:], den2[:])
            ratio = stp.tile([128, NBT], F32, name="ratio")
            nc.vector.tensor_mul(ratio[:], pos, rec[:])
            epsb = stp.tile([128, 1], F32, name="epsb")
            nc.vector.memset(epsb[:], float(EPS))
            lg = stp.tile([128, NBT], F32, name="lg")
            nc.scalar.activation(lg[:], ratio[:], ACTF.Ln, bias=epsb[:, 0:1])
            # partition sum via matmul; fold -1/B into the ones vector
            ones = stp.tile([128, 1], F32, name="ones")
            nc.vector.memset(ones[:], -1.0 / B)
            pl = psp.tile([128, MLOC], F32, name="pl", tag="ps")
            nc.tensor.matmul(pl[:1, 0:NBT], ones[:], lg[:])
            lout = stp.tile([1, 1], F32, name="lout")
            nc.vector.tensor_reduce(lout[:], pl[:1, 0:NBT], axis=AX.X, op=ALU.add)
            nc.sync.dma_start(lossd[:, :], lout[:])

    nc.compile()
    _PROG_CACHE[key] = nc
    return nc


def kernel(feats, feats_s, labels, labels_s, topk, num_instances):
    global LAST_RESULT
    feats = np.asarray(feats, dtype=np.float32)
    feats_s = np.asarray(feats_s, dtype=np.float32)
    labels = np.asarray(labels).astype(np.int64).ravel()
    labels_s = np.asarray(labels_s).astype(np.int64).ravel()
    tk, ni = int(topk), int(num_instances)
    assert feats.shape == (B, C), feats.shape
    assert tk * ni == G and feats_s.shape == (B, tk, C)

    Fs = feats_s.reshape(-1, C)                       # [16384, 256]
    glab = labels_s.reshape(KTOT, G)[:, 0]            # label of each block

    # rotation is valid if each core's own-label rows are exactly the
    # contiguous global rows [256j, 256j+256)
    rotate = True
    for j in range(NCORES):
        own = np.isin(labels, glab[j * KLOC:(j + 1) * KLOC])
        want = np.zeros(B, dtype=bool)
        want[j * (B // NCORES):(j + 1) * (B // NCORES)] = True
        if not np.array_equal(own, want):
            rotate = False
            break
    min_bts = (0, 1) if rotate else tuple(range(NBT))

    nc = _build(min_bts, rotate)

    in_maps = []
    for j in range(NCORES):
        shift = (B // NCORES) * j
        f_loc = np.roll(feats, -shift, axis=0) if rotate else feats
        lab_loc = np.roll(labels, -shift) if rotate else labels
        ftT = np.ascontiguousarray(f_loc.T).reshape(2, 128, B)
        fsT = np.ascontiguousarray(Fs[j * MLOC:(j + 1) * MLOC].T).reshape(2, 128, MLOC)
        # masks in local (rotated) coords: [p, bt*KLOC + k]
        lab2 = lab_loc.reshape(NBT, 128)                        # [bt, p]
        gl_j = glab[j * KLOC:(j + 1) * KLOC]                    # [KLOC]
        mp = (lab2[:, :, None] == gl_j[None, None, :])          # [bt, p, k]
        mp = mp.transpose(1, 0, 2).reshape(128, NBT * KLOC)
        in_maps.append({
            "featsT": ftT.astype(ml_dtypes.bfloat16),
            "fsT": fsT.astype(ml_dtypes.bfloat16),
            "mpos": mp.astype(np.float32),
            "mneg": (~mp).astype(np.float32),
        })

    LAST_RESULT = run_bass_kernel_spmd(nc, in_maps, core_ids=list(range(NCORES)))
    loss = LAST_RESULT.results[0]["loss"][0, 0]
    return np.asarray(loss, dtype=np.float32).reshape(())


# revision 7
# speedup vs baseline: 1.5016x; 1.1876x over previous
"""Trainium2 Bass kernel for nn_CriterionLP (LP contrastive criterion loss).

Reference computation (B=2048 anchors, M=16384 supports, C=256, K=128 label
groups of G=128 supports each):
    sim   = (feats @ Fs.T) / TEMP                  [B, M]
    E     = exp(sim) grouped into K blocks of G    [B, K, G]
    pos   = exp(min sim over own-label block)      (one block per row)
    neg   = sum over other blocks of exp(max sim over block)
    loss  = mean_b( -log(pos/(pos+neg+eps) + eps) )

Sharding: support axis across 8 cores (16 groups / 2048 support rows per
core).  Each core loads the full feats (bf16, 1MB) plus its Fs shard (1MB),
computes per-block stats for its groups, and the per-row pos/neg partial
sums are combined with an on-device AllReduce; every core then computes the
identical final scalar loss.

Key tricks:
  - exp is monotonic: block min/max are taken on raw matmul scores; exp only
    runs on the [128, 256] block-stat arrays.
  - per-core row rotation puts each core's own-label (positive) rows into
    b-tiles 0..1, so the min-reduction only runs on 2 of 16 b-tiles with a
    core-uniform program; partial sums are un-rotated with a dynamic-slice
    DMA (offset = f(partition_id)) before the AllReduce.
  - PSUM -> SBUF bf16 copies on the scalar engine; block max via pairwise
    tensor_tensor max tree on the vector engine (bf16 2x mode; plain
    tensor_reduce only runs at 1x).
"""

import numpy as np
import ml_dtypes

import concourse.bass as bass
import concourse.bacc as bacc
import concourse.tile as tile
import concourse.mybir as mybir
from concourse.bass_utils import run_bass_kernel_spmd

F32 = mybir.dt.float32
BF16 = mybir.dt.bfloat16
AX = mybir.AxisListType
ALU = mybir.AluOpType
ACTF = mybir.ActivationFunctionType

TEMP = 0.05
EPS = 1e-6
B, C = 2048, 256
NCORES = 8
KTOT, G = 128, 128          # label groups, supports per group
MLOC = 2048                 # support rows per core
KLOC = KTOT // NCORES       # groups per core (16)
NBT = B // 128              # b tiles of 128 rows (16)
NMT = MLOC // 512           # m tiles of 512 cols (4)

_PROG_CACHE = {}
LAST_RESULT = None          # BassKernelResults of the most recent run


def _tree(nc, pool, cpv, out16, op):
    """Block reduce [128, 16, 128] -> [128, 16] via pairwise TT ops (bf16 2x)."""
    t1 = pool.tile([128, KLOC, 64], BF16, name="t1", tag="t1", bufs=3)
    nc.vector.tensor_tensor(t1[:], cpv[:, :, 0:64], cpv[:, :, 64:128], op)
    t2 = pool.tile([128, KLOC, 32], BF16, name="t2", tag="t2", bufs=3)
    nc.vector.tensor_tensor(t2[:], t1[:, :, 0:32], t1[:, :, 32:64], op)
    t3 = pool.tile([128, KLOC, 16], BF16, name="t3", tag="t3", bufs=3)
    nc.vector.tensor_tensor(t3[:], t2[:, :, 0:16], t2[:, :, 16:32], op)
    t4 = pool.tile([128, KLOC, 8], BF16, name="t4", tag="t4", bufs=3)
    nc.vector.tensor_tensor(t4[:], t3[:, :, 0:8], t3[:, :, 8:16], op)
    nc.vector.tensor_reduce(out16, t4[:], axis=AX.X, op=op)


def _build(min_bts, rotate):
    key = (tuple(sorted(min_bts)), rotate)
    if key in _PROG_CACHE:
        return _PROG_CACHE[key]

    nc = bacc.Bacc("TRN2", target_bir_lowering=False, debug=False,
                   num_devices=NCORES)
    ftd = nc.dram_tensor("featsT", [2, 128, B], BF16, kind="ExternalInput")
    fsd = nc.dram_tensor("fsT", [2, 128, MLOC], BF16, kind="ExternalInput")
    mpd = nc.dram_tensor("mpos", [128, NBT * KLOC], F32, kind="ExternalInput")
    mnd = nc.dram_tensor("mneg", [128, NBT * KLOC], F32, kind="ExternalInput")
    lossd = nc.dram_tensor("loss", [1, 1], F32, kind="ExternalOutput")

    with tile.TileContext(nc) as tc:
        with (
            tc.tile_pool(name="wpool", bufs=1) as wp,
            tc.tile_pool(name="cpool", bufs=3) as cpp,
            tc.tile_pool(name="tpool", bufs=3) as trp,
            tc.tile_pool(name="spool", bufs=1) as stp,
            tc.tile_pool(name="pspool", bufs=2, space="PSUM") as psp,
            tc.tile_pool(name="drpool", bufs=1, space="DRAM") as drp,
        ):
            # warm-up collective FIRST: the collectives entry barrier only
            # starts once each core's queue reaches its first collective, so
            # putting a tiny AllReduce at the very top lets the cross-core
            # launch-skew sync overlap with the whole compute phase
            dmy = stp.tile([128, 1], F32, name="dmy")
            nc.vector.memset(dmy[:], 0.0)
            dmy_in = drp.tile([128, 1], F32, name="dmy_in")
            dmy_out = drp.tile([128, 1], F32, name="dmy_out", addr_space="Shared")
            nc.sync.dma_start(dmy_in[:, :], dmy[:])
            nc.gpsimd.collective_compute(
                "AllReduce", ALU.add,
                replica_groups=[list(range(NCORES))],
                ins=[dmy_in[:, :].opt()],
                outs=[dmy_out[:, :].opt()],
            )

            # --- input loads: fs via HWDGE(SP), ft via SWDGE(gpsimd) so the
            # two streams drain in parallel and compute can start early ---
            fs_sb = []
            for mt in range(NMT):
                t = wp.tile([128, 2, 512], BF16, name=f"fs{mt}", tag=f"fs{mt}")
                for ch in range(2):
                    nc.sync.dma_start(t[:, ch, :], fsd[ch, :, mt * 512:(mt + 1) * 512])
                fs_sb.append(t)
            ft_sb = []
            for bt in range(NBT):
                t = wp.tile([128, 2, 128], BF16, name=f"ft{bt}", tag=f"ft{bt}")
                for ch in range(2):
                    nc.gpsimd.dma_start(t[:, ch, :], ftd[ch, :, bt * 128:(bt + 1) * 128])
                ft_sb.append(t)
            mpos = stp.tile([128, NBT * KLOC], F32, name="mpos_sb")
            nc.gpsimd.dma_start(mpos[:], mpd[:, :])
            mneg = stp.tile([128, NBT * KLOC], F32, name="mneg_sb")
            nc.gpsimd.dma_start(mneg[:], mnd[:, :])

            blkmax = stp.tile([128, NBT * KLOC], BF16, name="blkmax")
            blkmin = stp.tile([128, NBT * KLOC], BF16, name="blkmin")
            # untouched min slots stay 0 -> exp(0)=1, masked to 0 by mpos
            nc.vector.memset(blkmin[:], 0.0)

            # --- main loop: matmul -> bf16 copy -> block max (+min) ---
            for bt in range(NBT):
                ps = psp.tile([128, MLOC], F32, name="ps", tag="ps")
                for ch in range(2):
                    for mt in range(NMT):
                        nc.tensor.matmul(
                            ps[:, mt * 512:(mt + 1) * 512],
                            ft_sb[bt][:, ch, :],
                            fs_sb[mt][:, ch, :],
                            start=(ch == 0),
                            stop=(ch == 1),
                        )
                cp = cpp.tile([128, MLOC], BF16, name="cp", tag="cp")
                nc.scalar.copy(cp[:], ps[:])
                cpv = cp.rearrange("p (k g) -> p k g", g=G)
                _tree(nc, trp, cpv, blkmax[:, bt * KLOC:(bt + 1) * KLOC], ALU.max)
                if bt in min_bts:
                    _tree(nc, trp, cpv, blkmin[:, bt * KLOC:(bt + 1) * KLOC], ALU.min)

            # --- epilogue: exp, masked sums ---
            emin = stp.tile([128, NBT * KLOC], F32, name="emin")
            nc.scalar.activation(emin[:], blkmin[:], ACTF.Exp, scale=1.0 / TEMP)
            emax = stp.tile([128, NBT * KLOC], F32, name="emax")
            nc.scalar.activation(emax[:], blkmax[:], ACTF.Exp, scale=1.0 / TEMP)
            prodp = stp.tile([128, NBT * KLOC], F32, name="prodp")
            nc.vector.tensor_mul(prodp[:], emin[:], mpos[:])
            prodn = stp.tile([128, NBT * KLOC], F32, name="prodn")
            nc.vector.tensor_mul(prodn[:], emax[:], mneg[:])

            # pack2: [p, t(=bt slot, doubled), side]; write once, duplicate.
            # bf16 payload halves the AllReduce transfer time.
            pack2 = stp.tile([128, 2 * NBT, 2], BF16, name="pack2")
            with nc.allow_low_precision("bf16 collective payload; 0.4% rel on sums is fine"):
                nc.vector.tensor_reduce(
                    pack2[:, 0:NBT, 0],
                    prodp.rearrange("p (t k) -> p t k", k=KLOC),
                    axis=AX.X, op=ALU.add)
                nc.vector.tensor_reduce(
                    pack2[:, 0:NBT, 1],
                    prodn.rearrange("p (t k) -> p t k", k=KLOC),
                    axis=AX.X, op=ALU.add)

            cc_in = drp.tile([128, NBT, 2], BF16, name="cc_in")
            cc_out = drp.tile([128, NBT, 2], BF16, name="cc_out", addr_space="Shared")
            if rotate:
                nc.vector.tensor_copy(pack2[:, NBT:2 * NBT, :], pack2[:, 0:NBT, :])
                # local bt -> global bt is a rotation by 2*pid; un-rotate by
                # reading a dynamic window of the doubled buffer
                pid = nc.partition_id(engines=[mybir.EngineType.SP])
                w = NBT - 2 * pid
                nc.sync.dma_start(cc_in[:, :, :], pack2[:, bass.ds(w, NBT), :])
            else:
                nc.sync.dma_start(cc_in[:, :, :], pack2[:, 0:NBT, :])

            nc.gpsimd.collective_compute(
                "AllReduce", ALU.add,
                replica_groups=[list(range(NCORES))],
                ins=[cc_in[:, :, :].opt()],
                outs=[cc_out[:, :, :].opt()],
            )

            red = stp.tile([128, NBT, 2], BF16, name="red")
            nc.sync.dma_start(red[:, :, :], cc_out[:, :, :])

            # --- final loss: -mean(log(pos/(pos+neg+eps)+eps)) ---
            pos = red[:, :, 0]
            neg = red[:, :, 1]
            den2 = stp.tile([128, NBT], F32, name="den2")
            nc.vector.scalar_tensor_tensor(
                den2[:], pos, float(EPS), neg, ALU.add, ALU.add)
            rec = stp.tile([128, NBT], F32, name="rec")
            nc.vector.reciprocal(rec[:], den2[:])
            ratio = stp.tile([128, NBT], F32, name="ratio")
            nc.vector.tensor_mul(ratio[:], pos, rec[:])
            epsb = stp.tile([128, 1], F32, name="epsb")
            nc.vector.memset(epsb[:], float(EPS))
            lg = stp.tile([128, NBT], F32, name="lg")
            nc.scalar.activation(lg[:], ratio[:], ACTF.Ln, bias=epsb[:, 0:1])
            # partition sum via matmul; fold -1/B into the ones vector
            ones = stp.tile([128, 1], F32, name="ones")
            nc.vector.memset(ones[:], -1.0 / B)
            pl = psp.tile([128, MLOC], F32, name="pl", tag="ps")
            nc.tensor.matmul(pl[:1, 0:NBT], ones[:], lg[:])
            lout = stp.tile([1, 1], F32, name="lout")
            nc.vector.tensor_reduce(lout[:], pl[:1, 0:NBT], axis=AX.X, op=ALU.add)
            nc.sync.dma_start(lossd[:, :], lout[:])

    nc.compile()
    _PROG_CACHE[key] = nc
    return nc


def kernel(feats, feats_s, labels, labels_s, topk, num_instances):
    global LAST_RESULT
    feats = np.asarray(feats, dtype=np.float32)
    feats_s = np.asarray(feats_s, dtype=np.float32)
    labels = np.asarray(labels).astype(np.int64).ravel()
    labels_s = np.asarray(labels_s).astype(np.int64).ravel()
    tk, ni = int(topk), int(num_instances)
    assert feats.shape == (B, C), feats.shape
    assert tk * ni == G and feats_s.shape == (B, tk, C)

    Fs = feats_s.reshape(-1, C)                       # [16384, 256]
    glab = labels_s.reshape(KTOT, G)[:, 0]            # label of each block

    # rotation is valid if each core's own-label rows are exactly the
    # contiguous global rows [256j, 256j+256)
    rotate = True
    for j in range(NCORES):
        own = np.isin(labels, glab[j * KLOC:(j + 1) * KLOC])
        want = np.zeros(B, dtype=bool)
        want[j * (B // NCORES):(j + 1) * (B // NCORES)] = True
        if not np.array_equal(own, want):
            rotate = False
            break
    min_bts = (0, 1) if rotate else tuple(range(NBT))

    nc = _build(min_bts, rotate)

    in_maps = []
    for j in range(NCORES):
        shift = (B // NCORES) * j
        f_loc = np.roll(feats, -shift, axis=0) if rotate else feats
        lab_loc = np.roll(labels, -shift) if rotate else labels
        ftT = np.ascontiguousarray(f_loc.T).reshape(2, 128, B)
        fsT = np.ascontiguousarray(Fs[j * MLOC:(j + 1) * MLOC].T).reshape(2, 128, MLOC)
        # masks in local (rotated) coords: [p, bt*KLOC + k]
        lab2 = lab_loc.reshape(NBT, 128)                        # [bt, p]
        gl_j = glab[j * KLOC:(j + 1) * KLOC]                    # [KLOC]
        mp = (lab2[:, :, None] == gl_j[None, None, :])          # [bt, p, k]
        mp = mp.transpose(1, 0, 2).reshape(128, NBT * KLOC)
        in_maps.append({
            "featsT": ftT.astype(ml_dtypes.bfloat16),
            "fsT": fsT.astype(ml_dtypes.bfloat16),
            "mpos": mp.astype(np.float32),
            "mneg": (~mp).astype(np.float32),
        })

    LAST_RESULT = run_bass_kernel_spmd(nc, in_maps, core_ids=list(range(NCORES)))
    loss = LAST_RESULT.results[0]["loss"][0, 0]
    return np.asarray(loss, dtype=np.float32).reshape(())
